# revision 8
# baseline (speedup 1.0000x reference)
"""CommNet message-passing kernel for Trainium2 (8 NeuronCores).

Problem (reference semantics):
    A, B, S, H = 8, 64, 1024, 128
    msg   = transpose(rnn_h, (2,1,0,3)) * alive            # (A,B,S,H)
    denom = max(sum_a alive, 1)                            # (1,B,S,1)
    msg   = msg / denom
    msg   = einsum('absh,oh->abso', msg, W) + b            # per-token HxH linear
    out   = obs + msg.reshape(A*B, S, H)

Sharding: data-parallel over the env-batch axis B (8 batches per core).
All ops are batch-local; W/b are replicated.

Per-core kernel layout strategy:
  - tokens are ordered (a, b, s) to match obs/out memory order, so the obs
    load and out store are contiguous; the rnn_h load is the strided stream
    (512B chunks) and implements the (S,B,A,H)->(A,B,S,H) permute.
  - per (a,b) pair: 1024 tokens = 8 sub-tiles of 128 tokens.
    Each 512-token group: pre-scale by alive/denom (DVE per-partition
    scalars), PE-transpose to (H, tokens), one W-stationary float32r matmul
    (N=512 -> full rate), bias added on ScalarE during the PSUM->SBUF copy,
    PE-transpose back to (tokens, H), residual add with obs on DVE.
  - alive -> scale = alive/max(sum_a alive,1) is computed on device with two
    small selector matmuls (partition-axis reduce + broadcast) and 8 PE
    transposes into the (token-partition, tile-column) layout.
"""

import os
import sys

import numpy as np

for _p in ("/opt/trn_rl_repo", "/root/.axon_site/_ro/trn_rl_repo"):
    if os.path.isdir(_p) and _p not in sys.path:
        sys.path.append(_p)

A, B, S, H = 8, 64, 1024, 128
NCORES = 8
BLOC = B // NCORES  # 8 env batches per core

F32 = None  # set lazily after imports


def _build_program(s_len=S, transpose_dt="float32"):
    """Build the per-core Bass program (identical on all cores)."""
    import concourse.bass as bass  # noqa: F401
    import concourse.bacc as bacc
    import concourse.tile as tile
    from concourse import mybir

    f32 = mybir.dt.float32
    f32r = mybir.dt.float32r
    i32 = mybir.dt.int32

    assert s_len % 512 == 0
    nj = s_len // 128       # sub-tiles per (a,b)
    ngroups = s_len // 512  # 512-token groups per (a,b)

    nc = bacc.Bacc("TRN2", target_bir_lowering=False, debug=False,
                   num_devices=NCORES)

    rnn = nc.dram_tensor("rnn", [s_len, BLOC, A, H], f32,
                         kind="ExternalInput").ap()
    obs = nc.dram_tensor("obs", [A, BLOC, s_len, H], f32,
                         kind="ExternalInput").ap()
    alive = nc.dram_tensor("alive", [A, BLOC, s_len], i32,
                           kind="ExternalInput").ap()
    wt = nc.dram_tensor("wt", [H, H], f32, kind="ExternalInput").ap()
    bias = nc.dram_tensor("bias", [H, 1], f32, kind="ExternalInput").ap()
    ident = nc.dram_tensor("ident", [128, 128], f32, kind="ExternalInput").ap()
    sel = nc.dram_tensor("sel", [64, 8], f32, kind="ExternalInput").ap()
    sel2 = nc.dram_tensor("sel2", [8, 64], f32, kind="ExternalInput").ap()
    out = nc.dram_tensor("out", [A, BLOC, s_len, H], f32,
                         kind="ExternalOutput").ap()

    # tokens (p within sub-tile j) views
    rnn_r = rnn.rearrange("(j p) b a h -> b a p j h", p=128)
    obs_r = obs.rearrange("a b (j p) h -> a b p j h", p=128)
    out_r = out.rearrange("a b (j p) h -> a b p j h", p=128)
    alive_r = alive.rearrange("a b s -> (a b) s")

    tr_cast = (lambda ap: ap.bitcast(f32r)) if transpose_dt == "float32r" \
        else (lambda ap: ap)

    with tile.TileContext(nc) as tc:
        with tc.tile_pool(name="consts", bufs=1) as consts, \
             tc.tile_pool(name="pre", bufs=1) as pre, \
             tc.tile_pool(name="prepsum", bufs=1, space="PSUM") as prepsum, \
             tc.tile_pool(name="rnnp", bufs=3) as rnn_pool, \
             tc.tile_pool(name="obsp", bufs=3) as obs_pool, \
             tc.tile_pool(name="outp", bufs=3) as out_pool, \
             tc.tile_pool(name="scaledp", bufs=3) as scaled_pool, \
             tc.tile_pool(name="mtp", bufs=3) as mt_pool, \
             tc.tile_pool(name="obp", bufs=3) as ob_pool, \
             tc.tile_pool(name="pap", bufs=2, space="PSUM") as pa_pool, \
             tc.tile_pool(name="pbp", bufs=2, space="PSUM") as pb_pool, \
             tc.tile_pool(name="pcp", bufs=2, space="PSUM") as pc_pool:

            # ---- constants ----
            wt_sb = consts.tile([128, 128], f32, tag="wt")
            nc.sync.dma_start(out=wt_sb, in_=wt)
            # fp32r matmul operands must be *produced* as float32r (walrus
            # verifier); round W once on DVE.
            wt_r = consts.tile([128, 128], f32r, tag="wtr")
            nc.vector.tensor_copy(out=wt_r, in_=wt_sb)
            id_sb = consts.tile([128, 128], f32, tag="id")
            nc.sync.dma_start(out=id_sb, in_=ident)
            b_sb = consts.tile([128, 1], f32, tag="b")
            nc.sync.dma_start(out=b_sb, in_=bias)
            sel_sb = consts.tile([64, 8], f32, tag="sel")
            nc.sync.dma_start(out=sel_sb, in_=sel)
            sel2_sb = consts.tile([8, 64], f32, tag="sel2")
            nc.sync.dma_start(out=sel2_sb, in_=sel2)

            # ---- scale = alive / max(sum_a alive, 1) ----
            alive_i = pre.tile([64, s_len], i32, tag="alive_i")
            nc.sync.dma_start(out=alive_i, in_=alive_r)
            alive_f = pre.tile([64, s_len], f32, tag="alive_f")
            nc.vector.tensor_copy(out=alive_f, in_=alive_i)

            denom = pre.tile([8, s_len], f32, tag="denom")
            for hh in range(s_len // 512):
                dps = prepsum.tile([8, 512], f32, tag="pp")
                nc.tensor.matmul(out=dps, lhsT=sel_sb,
                                 rhs=alive_f[:, 512 * hh:512 * (hh + 1)],
                                 start=True, stop=True)
                nc.vector.tensor_scalar_max(
                    out=denom[:, 512 * hh:512 * (hh + 1)], in0=dps,
                    scalar1=1.0)
            inv = pre.tile([8, s_len], f32, tag="inv")
            nc.vector.reciprocal(out=inv, in_=denom)

            scale_nat = pre.tile([64, s_len], f32, tag="scale_nat")
            for hh in range(s_len // 512):
                ips = prepsum.tile([64, 512], f32, tag="pp")
                nc.tensor.matmul(out=ips, lhsT=sel2_sb,
                                 rhs=inv[:, 512 * hh:512 * (hh + 1)],
                                 start=True, stop=True)
                nc.vector.tensor_mul(
                    out=scale_nat[:, 512 * hh:512 * (hh + 1)],
                    in0=alive_f[:, 512 * hh:512 * (hh + 1)], in1=ips)

            # scale_sb[p, 64*j + (a*8+b)] = scale for token (a, b, 128*j+p)
            scps = prepsum.tile([128, 64 * nj], f32, tag="pp")
            for c in range(nj):
                nc.tensor.matmul(out=scps[:, 64 * c:64 * (c + 1)],
                                 lhsT=scale_nat[:, 128 * c:128 * (c + 1)],
                                 rhs=id_sb[:64, :64], is_transpose=True,
                                 start=(c == 0), stop=(c == nj - 1))
            scale_sb = pre.tile([128, 64 * nj], f32, tag="scale_sb")
            nc.vector.tensor_copy(out=scale_sb, in_=scps)

            # ---- main loop over (a, b) pairs ----
            ident_f = mybir.ActivationFunctionType.Identity
            for a in range(A):
                for b in range(BLOC):
                    ab = a * 8 + b
                    rnn_t = rnn_pool.tile([128, nj, 128], f32, tag="rnn_t")
                    nc.sync.dma_start(out=rnn_t, in_=rnn_r[b, a])
                    obs_t = obs_pool.tile([128, nj, 128], f32, tag="obs_t")
                    nc.sync.dma_start(out=obs_t, in_=obs_r[a, b])
                    out_t = out_pool.tile([128, nj, 128], f32, tag="out_t")

                    obs_fl = obs_t.rearrange("p j h -> p (j h)")
                    out_fl = out_t.rearrange("p j h -> p (j h)")

                    for g in range(ngroups):
                        scaled = scaled_pool.tile([128, 4, 128], f32,
                                                  tag="scaled")
                        for jj in range(4):
                            j = 4 * g + jj
                            col = 64 * j + ab
                            nc.vector.tensor_scalar_mul(
                                out=scaled[:, jj, :], in0=rnn_t[:, j, :],
                                scalar1=scale_sb[:, col:col + 1])
                        pa = pa_pool.tile([128, 512], f32, tag="pa")
                        for jj in range(4):
                            nc.tensor.matmul(
                                out=tr_cast(pa[:, 128 * jj:128 * (jj + 1)]),
                                lhsT=tr_cast(scaled[:, jj, :]),
                                rhs=tr_cast(id_sb),
                                is_transpose=True,
                                start=(jj == 0), stop=(jj == 3))
                        mt = mt_pool.tile([128, 512], f32r, tag="mt")
                        nc.scalar.copy(out=mt, in_=pa)
                        pb = pb_pool.tile([128, 512], f32, tag="pb")
                        nc.tensor.matmul(out=pb, lhsT=wt_r, rhs=mt,
                                         start=True, stop=True)
                        ob = ob_pool.tile([128, 512], f32, tag="ob")
                        nc.scalar.activation(out=ob, in_=pb, func=ident_f,
                                             bias=b_sb, scale=1.0)
                        pc = pc_pool.tile([128, 512], f32, tag="pc")
                        for jj in range(4):
                            nc.tensor.matmul(
                                out=tr_cast(pc[:, 128 * jj:128 * (jj + 1)]),
                                lhsT=tr_cast(ob[:, 128 * jj:128 * (jj + 1)]),
                                rhs=tr_cast(id_sb),
                                is_transpose=True,
                                start=(jj == 0), stop=(jj == 3))
                        nc.vector.tensor_add(
                            out=out_fl[:, 512 * g:512 * (g + 1)], in0=pc,
                            in1=obs_fl[:, 512 * g:512 * (g + 1)])
                    nc.sync.dma_start(out=out_r[a, b], in_=out_t)
    nc.compile()
    return nc


def make_in_maps(obs, rnn_h, alive, W, b, s_len=S):
    """Shard full inputs into per-core input maps (host-side slicing only)."""
    obs4 = obs.reshape(A, B, S, H)
    wt = np.ascontiguousarray(W.T.astype(np.float32))
    b2 = np.ascontiguousarray(b.astype(np.float32).reshape(H, 1))
    ident = np.eye(128, dtype=np.float32)
    sel = np.zeros((64, 8), np.float32)
    sel[np.arange(64), np.arange(64) % 8] = 1.0
    sel2 = np.ascontiguousarray(sel.T)
    in_maps = []
    for c in range(NCORES):
        bs = slice(BLOC * c, BLOC * (c + 1))
        in_maps.append({
            "rnn": np.ascontiguousarray(rnn_h[:s_len, bs]),
            "obs": np.ascontiguousarray(obs4[:, bs, :s_len]),
            "alive": np.ascontiguousarray(alive[:, bs, :s_len, 0]),
            "wt": wt, "bias": b2, "ident": ident, "sel": sel, "sel2": sel2,
        })
    return in_maps


_NC_CACHE = {}


def get_nc(s_len=S, transpose_dt="float32"):
    key = (s_len, transpose_dt)
    if key not in _NC_CACHE:
        _NC_CACHE[key] = _build_program(s_len, transpose_dt)
    return _NC_CACHE[key]


def kernel(obs, rnn_h, alive, W, b):
    from concourse.bass_utils import run_bass_kernel_spmd

    nc = get_nc()
    in_maps = make_in_maps(obs, rnn_h, alive, W, b)
    res = run_bass_kernel_spmd(nc, in_maps, list(range(NCORES))).results
    out = np.empty((A, B, S, H), np.float32)
    for c in range(NCORES):
        out[:, BLOC * c:BLOC * (c + 1)] = res[c]["out"]
    return out.reshape(A * B, S, H)


# revision 11
# speedup vs baseline: 180.7280x; 180.7280x over previous
"""CommNet message-passing kernel for Trainium2 (8 NeuronCores).

Problem (reference semantics):
    A, B, S, H = 8, 64, 1024, 128
    msg   = transpose(rnn_h, (2,1,0,3)) * alive            # (A,B,S,H)
    denom = max(sum_a alive, 1)                            # (1,B,S,1)
    msg   = msg / denom
    msg   = einsum('absh,oh->abso', msg, W) + b            # per-token HxH linear
    out   = obs + msg.reshape(A*B, S, H)

Sharding: data-parallel over the env-batch axis B (8 batches per core).
All ops are batch-local; W/b are replicated.

Per-core kernel layout strategy:
  - tokens are ordered (a, b, s) to match obs/out memory order, so the obs
    load and out store are contiguous; the rnn_h load is the strided stream
    (512B chunks) and implements the (S,B,A,H)->(A,B,S,H) permute.
  - per (a,b) pair: 1024 tokens = 8 sub-tiles of 128 tokens.
    Each 512-token group: pre-scale by alive/denom (DVE per-partition
    scalars), PE-transpose to (H, tokens), one W-stationary float32r matmul
    (N=512 -> full rate), bias added on ScalarE during the PSUM->SBUF copy,
    PE-transpose back to (tokens, H), residual add with obs on DVE.
  - alive -> scale = alive/max(sum_a alive,1) is computed on device with two
    small selector matmuls (partition-axis reduce + broadcast) and 8 PE
    transposes into the (token-partition, tile-column) layout.
"""

import os
import sys

import numpy as np

for _p in ("/opt/trn_rl_repo", "/root/.axon_site/_ro/trn_rl_repo"):
    if os.path.isdir(_p) and _p not in sys.path:
        sys.path.append(_p)

A, B, S, H = 8, 64, 1024, 128
NCORES = 8
BLOC = B // NCORES  # 8 env batches per core

F32 = None  # set lazily after imports


def _build_program(s_len=S, transpose_dt="float32", reps=1):
    """Build the per-core Bass program (identical on all cores).

    reps>1 repeats the whole main loop (same I/O) — used only for timing,
    since single-call wall time is dominated by ~70ms axon RTT."""
    import concourse.bass as bass  # noqa: F401
    import concourse.bacc as bacc
    import concourse.tile as tile
    from concourse import mybir

    f32 = mybir.dt.float32
    f32r = mybir.dt.float32r
    i32 = mybir.dt.int32

    assert s_len % 512 == 0
    nj = s_len // 128       # sub-tiles per (a,b)
    ngroups = s_len // 512  # 512-token groups per (a,b)

    nc = bacc.Bacc("TRN2", target_bir_lowering=False, debug=False,
                   num_devices=NCORES)

    rnn = nc.dram_tensor("rnn", [s_len, BLOC, A, H], f32,
                         kind="ExternalInput").ap()
    obs = nc.dram_tensor("obs", [A, BLOC, s_len, H], f32,
                         kind="ExternalInput").ap()
    alive = nc.dram_tensor("alive", [A, BLOC, s_len], i32,
                           kind="ExternalInput").ap()
    wt = nc.dram_tensor("wt", [H, H], f32, kind="ExternalInput").ap()
    bias = nc.dram_tensor("bias", [H, 1], f32, kind="ExternalInput").ap()
    ident = nc.dram_tensor("ident", [128, 128], f32, kind="ExternalInput").ap()
    sel = nc.dram_tensor("sel", [64, 8], f32, kind="ExternalInput").ap()
    sel2 = nc.dram_tensor("sel2", [8, 64], f32, kind="ExternalInput").ap()
    out = nc.dram_tensor("out", [A, BLOC, s_len, H], f32,
                         kind="ExternalOutput").ap()

    # tokens (p within sub-tile j) views
    rnn_r = rnn.rearrange("(j p) b a h -> b a p j h", p=128)
    obs_r = obs.rearrange("a b (j p) h -> a b p j h", p=128)
    out_r = out.rearrange("a b (j p) h -> a b p j h", p=128)
    alive_r = alive.rearrange("a b s -> (a b) s")

    tr_cast = (lambda ap: ap.bitcast(f32r)) if transpose_dt == "float32r" \
        else (lambda ap: ap)

    with tile.TileContext(nc) as tc:
        with tc.tile_pool(name="consts", bufs=1) as consts, \
             tc.tile_pool(name="pre", bufs=1) as pre, \
             tc.tile_pool(name="prepsum", bufs=1, space="PSUM") as prepsum, \
             tc.tile_pool(name="rnnp", bufs=3) as rnn_pool, \
             tc.tile_pool(name="obsp", bufs=3) as obs_pool, \
             tc.tile_pool(name="outp", bufs=3) as out_pool, \
             tc.tile_pool(name="scaledp", bufs=3) as scaled_pool, \
             tc.tile_pool(name="mtp", bufs=3) as mt_pool, \
             tc.tile_pool(name="obp", bufs=3) as ob_pool, \
             tc.tile_pool(name="pap", bufs=2, space="PSUM") as pa_pool, \
             tc.tile_pool(name="pbp", bufs=2, space="PSUM") as pb_pool, \
             tc.tile_pool(name="pcp", bufs=2, space="PSUM") as pc_pool:

            # ---- constants ----
            wt_sb = consts.tile([128, 128], f32, tag="wt")
            nc.sync.dma_start(out=wt_sb, in_=wt)
            # fp32r matmul operands must be *produced* as float32r (walrus
            # verifier); round W once on DVE.
            wt_r = consts.tile([128, 128], f32r, tag="wtr")
            nc.vector.tensor_copy(out=wt_r, in_=wt_sb)
            id_sb = consts.tile([128, 128], f32, tag="id")
            nc.sync.dma_start(out=id_sb, in_=ident)
            b_sb = consts.tile([128, 1], f32, tag="b")
            nc.sync.dma_start(out=b_sb, in_=bias)
            sel_sb = consts.tile([64, 8], f32, tag="sel")
            nc.sync.dma_start(out=sel_sb, in_=sel)
            sel2_sb = consts.tile([8, 64], f32, tag="sel2")
            nc.sync.dma_start(out=sel2_sb, in_=sel2)

            # ---- scale = alive / max(sum_a alive, 1) ----
            alive_i = pre.tile([64, s_len], i32, tag="alive_i")
            nc.sync.dma_start(out=alive_i, in_=alive_r)
            alive_f = pre.tile([64, s_len], f32, tag="alive_f")
            nc.vector.tensor_copy(out=alive_f, in_=alive_i)

            denom = pre.tile([8, s_len], f32, tag="denom")
            for hh in range(s_len // 512):
                dps = prepsum.tile([8, 512], f32, tag="pp")
                nc.tensor.matmul(out=dps, lhsT=sel_sb,
                                 rhs=alive_f[:, 512 * hh:512 * (hh + 1)],
                                 start=True, stop=True)
                nc.vector.tensor_scalar_max(
                    out=denom[:, 512 * hh:512 * (hh + 1)], in0=dps,
                    scalar1=1.0)
            inv = pre.tile([8, s_len], f32, tag="inv")
            nc.vector.reciprocal(out=inv, in_=denom)

            scale_nat = pre.tile([64, s_len], f32, tag="scale_nat")
            for hh in range(s_len // 512):
                ips = prepsum.tile([64, 512], f32, tag="pp")
                nc.tensor.matmul(out=ips, lhsT=sel2_sb,
                                 rhs=inv[:, 512 * hh:512 * (hh + 1)],
                                 start=True, stop=True)
                nc.vector.tensor_mul(
                    out=scale_nat[:, 512 * hh:512 * (hh + 1)],
                    in0=alive_f[:, 512 * hh:512 * (hh + 1)], in1=ips)

            # scale_sb[p, 64*j + (a*8+b)] = scale for token (a, b, 128*j+p)
            scps = prepsum.tile([128, 64 * nj], f32, tag="pp")
            for c in range(nj):
                nc.tensor.matmul(out=scps[:, 64 * c:64 * (c + 1)],
                                 lhsT=scale_nat[:, 128 * c:128 * (c + 1)],
                                 rhs=id_sb[:64, :64], is_transpose=True,
                                 start=(c == 0), stop=(c == nj - 1))
            scale_sb = pre.tile([128, 64 * nj], f32, tag="scale_sb")
            nc.vector.tensor_copy(out=scale_sb, in_=scps)

            # ---- main loop over (a, b) pairs ----
            ident_f = mybir.ActivationFunctionType.Identity
            for _rep in range(reps):
              for a in range(A):
                for b in range(BLOC):
                    ab = a * 8 + b
                    rnn_t = rnn_pool.tile([128, nj, 128], f32, tag="rnn_t")
                    nc.sync.dma_start(out=rnn_t, in_=rnn_r[b, a])
                    obs_t = obs_pool.tile([128, nj, 128], f32, tag="obs_t")
                    nc.sync.dma_start(out=obs_t, in_=obs_r[a, b])
                    out_t = out_pool.tile([128, nj, 128], f32, tag="out_t")

                    obs_fl = obs_t.rearrange("p j h -> p (j h)")
                    out_fl = out_t.rearrange("p j h -> p (j h)")

                    for g in range(ngroups):
                        scaled = scaled_pool.tile([128, 4, 128], f32,
                                                  tag="scaled")
                        for jj in range(4):
                            j = 4 * g + jj
                            col = 64 * j + ab
                            nc.vector.tensor_scalar_mul(
                                out=scaled[:, jj, :], in0=rnn_t[:, j, :],
                                scalar1=scale_sb[:, col:col + 1])
                        pa = pa_pool.tile([128, 512], f32, tag="pa")
                        for jj in range(4):
                            nc.tensor.matmul(
                                out=tr_cast(pa[:, 128 * jj:128 * (jj + 1)]),
                                lhsT=tr_cast(scaled[:, jj, :]),
                                rhs=tr_cast(id_sb),
                                is_transpose=True,
                                start=(jj == 0), stop=(jj == 3))
                        mt = mt_pool.tile([128, 512], f32r, tag="mt")
                        nc.scalar.copy(out=mt, in_=pa)
                        pb = pb_pool.tile([128, 512], f32, tag="pb")
                        nc.tensor.matmul(out=pb, lhsT=wt_r, rhs=mt,
                                         start=True, stop=True)
                        ob = ob_pool.tile([128, 512], f32, tag="ob")
                        nc.scalar.activation(out=ob, in_=pb, func=ident_f,
                                             bias=b_sb, scale=1.0)
                        pc = pc_pool.tile([128, 512], f32, tag="pc")
                        for jj in range(4):
                            nc.tensor.matmul(
                                out=tr_cast(pc[:, 128 * jj:128 * (jj + 1)]),
                                lhsT=tr_cast(ob[:, 128 * jj:128 * (jj + 1)]),
                                rhs=tr_cast(id_sb),
                                is_transpose=True,
                                start=(jj == 0), stop=(jj == 3))
                        nc.vector.tensor_add(
                            out=out_fl[:, 512 * g:512 * (g + 1)], in0=pc,
                            in1=obs_fl[:, 512 * g:512 * (g + 1)])
                    nc.sync.dma_start(out=out_r[a, b], in_=out_t)
    nc.compile()
    return nc


def make_in_maps(obs, rnn_h, alive, W, b, s_len=S):
    """Shard full inputs into per-core input maps (host-side slicing only)."""
    obs4 = obs.reshape(A, B, S, H)
    wt = np.ascontiguousarray(W.T.astype(np.float32))
    b2 = np.ascontiguousarray(b.astype(np.float32).reshape(H, 1))
    ident = np.eye(128, dtype=np.float32)
    sel = np.zeros((64, 8), np.float32)
    sel[np.arange(64), np.arange(64) % 8] = 1.0
    sel2 = np.ascontiguousarray(sel.T)
    in_maps = []
    for c in range(NCORES):
        bs = slice(BLOC * c, BLOC * (c + 1))
        in_maps.append({
            "rnn": np.ascontiguousarray(rnn_h[:s_len, bs]),
            "obs": np.ascontiguousarray(obs4[:, bs, :s_len]),
            "alive": np.ascontiguousarray(alive[:, bs, :s_len, 0]),
            "wt": wt, "bias": b2, "ident": ident, "sel": sel, "sel2": sel2,
        })
    return in_maps


_NC_CACHE = {}


def get_nc(s_len=S, transpose_dt="float32", reps=1):
    key = (s_len, transpose_dt, reps)
    if key not in _NC_CACHE:
        _NC_CACHE[key] = _build_program(s_len, transpose_dt, reps)
    return _NC_CACHE[key]


def kernel(obs, rnn_h, alive, W, b):
    from concourse.bass_utils import run_bass_kernel_spmd

    nc = get_nc()
    in_maps = make_in_maps(obs, rnn_h, alive, W, b)
    res = run_bass_kernel_spmd(nc, in_maps, list(range(NCORES))).results
    out = np.empty((A, B, S, H), np.float32)
    for c in range(NCORES):
        out[:, BLOC * c:BLOC * (c + 1)] = res[c]["out"]
    return out.reshape(A * B, S, H)


# revision 16
# speedup vs baseline: 240.9881x; 1.3334x over previous
"""CommNet message-passing kernel for Trainium2 (8 NeuronCores).

Problem (reference semantics):
    A, B, S, H = 8, 64, 1024, 128
    msg   = transpose(rnn_h, (2,1,0,3)) * alive            # (A,B,S,H)
    denom = max(sum_a alive, 1)                            # (1,B,S,1)
    msg   = msg / denom
    msg   = einsum('absh,oh->abso', msg, W) + b            # per-token HxH linear
    out   = obs + msg.reshape(A*B, S, H)

Sharding: data-parallel over the env-batch axis B (8 batches per core).
All ops are batch-local; W/b are replicated.

Per-core kernel layout strategy:
  - tokens are ordered (a, b, s) to match obs/out memory order, so the obs
    load and out store are contiguous; the rnn_h load is the strided stream
    (512B chunks) and implements the (S,B,A,H)->(A,B,S,H) permute.
  - per (a,b) pair: 1024 tokens = 8 sub-tiles of 128 tokens.
    Each 512-token group: pre-scale by alive/denom (DVE per-partition
    scalars), PE-transpose to (H, tokens), one W-stationary float32r matmul
    (N=512 -> full rate), bias added on ScalarE during the PSUM->SBUF copy,
    PE-transpose back to (tokens, H), residual add with obs on DVE.
  - alive -> scale = alive/max(sum_a alive,1) is computed on device with two
    small selector matmuls (partition-axis reduce + broadcast) and 8 PE
    transposes into the (token-partition, tile-column) layout.
"""

import os
import sys

import numpy as np

for _p in ("/opt/trn_rl_repo", "/root/.axon_site/_ro/trn_rl_repo"):
    if os.path.isdir(_p) and _p not in sys.path:
        sys.path.append(_p)

A, B, S, H = 8, 64, 1024, 128
NCORES = 8
BLOC = B // NCORES  # 8 env batches per core

F32 = None  # set lazily after imports


def _build_program(s_len=S, transpose_dt="float32", reps=1):
    """Build the per-core Bass program (identical on all cores).

    reps>1 repeats the whole main loop (same I/O) — used only for timing,
    since single-call wall time is dominated by ~70ms axon RTT."""
    import concourse.bass as bass  # noqa: F401
    import concourse.bacc as bacc
    import concourse.tile as tile
    from concourse import mybir

    f32 = mybir.dt.float32
    f32r = mybir.dt.float32r
    i32 = mybir.dt.int32

    assert s_len % 512 == 0
    nj = s_len // 128       # sub-tiles per (a,b)
    ngroups = s_len // 512  # 512-token groups per (a,b)

    nc = bacc.Bacc("TRN2", target_bir_lowering=False, debug=False,
                   num_devices=NCORES)

    rnn = nc.dram_tensor("rnn", [s_len, BLOC, A, H], f32,
                         kind="ExternalInput").ap()
    obs = nc.dram_tensor("obs", [A, BLOC, s_len, H], f32,
                         kind="ExternalInput").ap()
    alive = nc.dram_tensor("alive", [A, BLOC, s_len], i32,
                           kind="ExternalInput").ap()
    wt = nc.dram_tensor("wt", [H, H], f32, kind="ExternalInput").ap()
    bias = nc.dram_tensor("bias", [H, 1], f32, kind="ExternalInput").ap()
    ident = nc.dram_tensor("ident", [128, 128], f32, kind="ExternalInput").ap()
    sel = nc.dram_tensor("sel", [64, 8], f32, kind="ExternalInput").ap()
    sel2 = nc.dram_tensor("sel2", [8, 64], f32, kind="ExternalInput").ap()
    out = nc.dram_tensor("out", [A, BLOC, s_len, H], f32,
                         kind="ExternalOutput").ap()

    # tokens (p within sub-tile j) views
    rnn_r = rnn.rearrange("(j p) b a h -> b a p j h", p=128)
    obs_r = obs.rearrange("a b (j p) h -> a b p j h", p=128)
    out_r = out.rearrange("a b (j p) h -> a b p j h", p=128)
    alive_r = alive.rearrange("a b s -> (a b) s")

    # dtype used for the transpose path (scaled msg, pa, ob, pc tiles).
    # float32r streams 1.5 cycles/row through the PE vs 2.0 for float32;
    # values are identical bits (transpose is routing; the producers round).
    tdt = {"float32": f32, "float32r": f32r,
           "bfloat16": mybir.dt.bfloat16}[transpose_dt]
    mm_dt = mybir.dt.bfloat16 if transpose_dt == "bfloat16" else f32r

    with tile.TileContext(nc) as tc:
        with tc.tile_pool(name="consts", bufs=1) as consts, \
             tc.tile_pool(name="pre", bufs=1) as pre, \
             tc.tile_pool(name="prepsum", bufs=1, space="PSUM") as prepsum, \
             tc.tile_pool(name="rnnp", bufs=3) as rnn_pool, \
             tc.tile_pool(name="obsp", bufs=3) as obs_pool, \
             tc.tile_pool(name="outp", bufs=3) as out_pool, \
             tc.tile_pool(name="scaledp", bufs=3) as scaled_pool, \
             tc.tile_pool(name="mtp", bufs=3) as mt_pool, \
             tc.tile_pool(name="obp", bufs=3) as ob_pool, \
             tc.tile_pool(name="pap", bufs=2, space="PSUM") as pa_pool, \
             tc.tile_pool(name="pbp", bufs=2, space="PSUM") as pb_pool, \
             tc.tile_pool(name="pcp", bufs=2, space="PSUM") as pc_pool:

            # ---- constants ----
            wt_sb = consts.tile([128, 128], f32, tag="wt")
            nc.sync.dma_start(out=wt_sb, in_=wt)
            # fp32r matmul operands must be *produced* as float32r (walrus
            # verifier); round W once on DVE.
            wt_r = consts.tile([128, 128], mm_dt, tag="wtr")
            nc.vector.tensor_copy(out=wt_r, in_=wt_sb)
            id_sb = consts.tile([128, 128], f32, tag="id")
            nc.sync.dma_start(out=id_sb, in_=ident)
            b_sb = consts.tile([128, 1], f32, tag="b")
            nc.sync.dma_start(out=b_sb, in_=bias)
            sel_sb = consts.tile([64, 8], f32, tag="sel")
            nc.sync.dma_start(out=sel_sb, in_=sel)
            sel2_sb = consts.tile([8, 64], f32, tag="sel2")
            nc.sync.dma_start(out=sel2_sb, in_=sel2)
            if tdt == f32:
                id_t = id_sb
            else:
                id_t = consts.tile([128, 128], tdt, tag="idt")
                nc.vector.tensor_copy(out=id_t, in_=id_sb)

            # ---- scale = alive / max(sum_a alive, 1) ----
            alive_i = pre.tile([64, s_len], i32, tag="alive_i")
            nc.sync.dma_start(out=alive_i, in_=alive_r)
            alive_f = pre.tile([64, s_len], f32, tag="alive_f")
            nc.vector.tensor_copy(out=alive_f, in_=alive_i)

            denom = pre.tile([8, s_len], f32, tag="denom")
            for hh in range(s_len // 512):
                dps = prepsum.tile([8, 512], f32, tag="pp")
                nc.tensor.matmul(out=dps, lhsT=sel_sb,
                                 rhs=alive_f[:, 512 * hh:512 * (hh + 1)],
                                 start=True, stop=True)
                nc.vector.tensor_scalar_max(
                    out=denom[:, 512 * hh:512 * (hh + 1)], in0=dps,
                    scalar1=1.0)
            inv = pre.tile([8, s_len], f32, tag="inv")
            nc.vector.reciprocal(out=inv, in_=denom)

            scale_nat = pre.tile([64, s_len], f32, tag="scale_nat")
            for hh in range(s_len // 512):
                ips = prepsum.tile([64, 512], f32, tag="pp")
                nc.tensor.matmul(out=ips, lhsT=sel2_sb,
                                 rhs=inv[:, 512 * hh:512 * (hh + 1)],
                                 start=True, stop=True)
                nc.vector.tensor_mul(
                    out=scale_nat[:, 512 * hh:512 * (hh + 1)],
                    in0=alive_f[:, 512 * hh:512 * (hh + 1)], in1=ips)

            # scale_sb[p, 64*j + (a*8+b)] = scale for token (a, b, 128*j+p)
            scps = prepsum.tile([128, 64 * nj], f32, tag="pp")
            for c in range(nj):
                nc.tensor.matmul(out=scps[:, 64 * c:64 * (c + 1)],
                                 lhsT=scale_nat[:, 128 * c:128 * (c + 1)],
                                 rhs=id_sb[:64, :64], is_transpose=True,
                                 start=(c == 0), stop=(c == nj - 1))
            scale_sb = pre.tile([128, 64 * nj], f32, tag="scale_sb")
            nc.vector.tensor_copy(out=scale_sb, in_=scps)

            # ---- main loop over (a, b) pairs ----
            ident_f = mybir.ActivationFunctionType.Identity
            for _rep in range(reps):
              for a in range(A):
                for b in range(BLOC):
                    ab = a * 8 + b
                    rnn_t = rnn_pool.tile([128, nj, 128], f32, tag="rnn_t")
                    nc.sync.dma_start(out=rnn_t, in_=rnn_r[b, a])
                    obs_t = obs_pool.tile([128, nj, 128], f32, tag="obs_t")
                    nc.sync.dma_start(out=obs_t, in_=obs_r[a, b])
                    out_t = out_pool.tile([128, nj, 128], f32, tag="out_t")

                    obs_fl = obs_t.rearrange("p j h -> p (j h)")
                    out_fl = out_t.rearrange("p j h -> p (j h)")

                    for g in range(ngroups):
                        scaled = scaled_pool.tile([128, 4, 128], tdt,
                                                  tag="scaled")
                        for jj in range(4):
                            j = 4 * g + jj
                            col = 64 * j + ab
                            nc.vector.tensor_scalar_mul(
                                out=scaled[:, jj, :], in0=rnn_t[:, j, :],
                                scalar1=scale_sb[:, col:col + 1])
                        pa = pa_pool.tile([128, 512], tdt, tag="pa")
                        for jj in range(4):
                            nc.tensor.matmul(
                                out=pa[:, 128 * jj:128 * (jj + 1)],
                                lhsT=scaled[:, jj, :],
                                rhs=id_t,
                                is_transpose=True,
                                start=(jj == 0), stop=(jj == 3))
                        mt = mt_pool.tile([128, 512], mm_dt, tag="mt")
                        nc.scalar.copy(out=mt, in_=pa)
                        pb = pb_pool.tile([128, 512], f32, tag="pb")
                        nc.tensor.matmul(out=pb, lhsT=wt_r, rhs=mt,
                                         start=True, stop=True)
                        ob = ob_pool.tile([128, 512], tdt, tag="ob")
                        nc.scalar.activation(out=ob, in_=pb, func=ident_f,
                                             bias=b_sb, scale=1.0)
                        pc = pc_pool.tile([128, 512], tdt, tag="pc")
                        for jj in range(4):
                            nc.tensor.matmul(
                                out=pc[:, 128 * jj:128 * (jj + 1)],
                                lhsT=ob[:, 128 * jj:128 * (jj + 1)],
                                rhs=id_t,
                                is_transpose=True,
                                start=(jj == 0), stop=(jj == 3))
                        nc.vector.tensor_add(
                            out=out_fl[:, 512 * g:512 * (g + 1)], in0=pc,
                            in1=obs_fl[:, 512 * g:512 * (g + 1)])
                    nc.sync.dma_start(out=out_r[a, b], in_=out_t)
    nc.compile()
    return nc


def make_in_maps(obs, rnn_h, alive, W, b, s_len=S):
    """Shard full inputs into per-core input maps (host-side slicing only)."""
    obs4 = obs.reshape(A, B, S, H)
    wt = np.ascontiguousarray(W.T.astype(np.float32))
    b2 = np.ascontiguousarray(b.astype(np.float32).reshape(H, 1))
    ident = np.eye(128, dtype=np.float32)
    sel = np.zeros((64, 8), np.float32)
    sel[np.arange(64), np.arange(64) % 8] = 1.0
    sel2 = np.ascontiguousarray(sel.T)
    in_maps = []
    for c in range(NCORES):
        bs = slice(BLOC * c, BLOC * (c + 1))
        in_maps.append({
            "rnn": np.ascontiguousarray(rnn_h[:s_len, bs]),
            "obs": np.ascontiguousarray(obs4[:, bs, :s_len]),
            "alive": np.ascontiguousarray(alive[:, bs, :s_len, 0]),
            "wt": wt, "bias": b2, "ident": ident, "sel": sel, "sel2": sel2,
        })
    return in_maps


_NC_CACHE = {}


def get_nc(s_len=S, transpose_dt="float32", reps=1):
    key = (s_len, transpose_dt, reps)
    if key not in _NC_CACHE:
        _NC_CACHE[key] = _build_program(s_len, transpose_dt, reps)
    return _NC_CACHE[key]


def kernel(obs, rnn_h, alive, W, b):
    from concourse.bass_utils import run_bass_kernel_spmd

    nc = get_nc()
    in_maps = make_in_maps(obs, rnn_h, alive, W, b)
    res = run_bass_kernel_spmd(nc, in_maps, list(range(NCORES))).results
    out = np.empty((A, B, S, H), np.float32)
    for c in range(NCORES):
        out[:, BLOC * c:BLOC * (c + 1)] = res[c]["out"]
    return out.reshape(A * B, S, H)
